# revision 12
# baseline (speedup 1.0000x reference)
"""Causal single-head attention (B=4, S=4096, E=1024, D=128) on 8 TRN2 NeuronCores.

Sharding: core = (batch b, query-group g) with b = core//2, g = core%2.
Each core owns batch b and half of its query tiles, interleaved in pairs of
128-row tiles so causal work stays balanced: local pair m covers absolute
query tiles {4m+2g, 4m+2g+1} and attends to key tiles 0..4m+3.

Per-core device program (bf16 matmuls, fp32 accumulate):
  Q_T = Wq.T @ xq.T   [d=128, 2048]   (lhsT = Wq e-chunks, rhs = xqT)
  K_T = Wk.T @ x.T    [d=128, 4096]
  V   = x @ Wv        [4096, 128] as 32 partition-tiles, plus a ones column
                      so the attention matmul also accumulates the softmax
                      denominator l = sum_k exp(s).
  per pair m, key-tile group grp in 0..m (4 key tiles per group):
      s_T[4 tiles] -> one [128, 1024] PSUM supertile  (4 matmuls, N=256)
      += causal mask on the last group (per-core input: the same graph slot
         needs different masks per query-group)
      P_T = exp(s_T * D**-0.5) in one activation  (no row-max: scores ~N(0,1))
  per query tile c in pair:
      O = sum_t P_T[t][:,c].T @ [V[t] | 1]       [q=128, 129]    (PSUM)
      out[q,:] = O[:,0:128] * (1 / O[:,128])
"""

import sys
import types

import numpy as np
import ml_dtypes

B, S, E, D = 4, 4096, 1024, 128
N_CORES = 8
NPAIR = 8          # local query-tile pairs per core
PAIR_W = 256       # two 128-row query tiles
SQ = S // 2        # query rows per core
SCALE = float(D) ** -0.5
NEG = -1e30
N_WARMUP = 22      # PE warmup matmuls issued before any DMA-dependent work

_cache = {}


def _install_ntff_shim():
    """antenv.axon_hooks is missing in this image; recreate it so
    run_bass_kernel_spmd(trace=True) can capture NTFF profiles."""
    if "antenv.axon_hooks" in sys.modules:
        return
    try:
        from trn_agent_boot.trn_boot import _ntff_profile_via_ctypes

        hook = _ntff_profile_via_ctypes("/opt/axon/libaxon_pjrt.so")
    except Exception:
        hook = None
    mod = types.ModuleType("antenv.axon_hooks")
    mod.get_axon_ntff_profile_hook = lambda: hook
    mod.set_axon_ntff_profile_hook = lambda h: None
    sys.modules["antenv.axon_hooks"] = mod


def build_nc():
    from contextlib import ExitStack

    import concourse.tile as tile
    from concourse import bacc, mybir
    from concourse.bass import ds, ts
    from concourse.masks import make_identity

    f32 = mybir.dt.float32
    bf16 = mybir.dt.bfloat16

    nc = bacc.Bacc("TRN2", target_bir_lowering=False, debug=False)
    xt = nc.dram_tensor("xt", [128, 8, S], bf16, kind="ExternalInput").ap()
    xqt = nc.dram_tensor("xqt", [128, 8, SQ], bf16, kind="ExternalInput").ap()
    w_all = nc.dram_tensor("w_all", [128, 3, 8, D], bf16, kind="ExternalInput").ap()
    mask = nc.dram_tensor("mask", [128, 1024], bf16, kind="ExternalInput").ap()
    out = nc.dram_tensor("out", [SQ, D], f32, kind="ExternalOutput").ap()

    with tile.TileContext(nc) as tc, ExitStack() as ctx:
        consts = ctx.enter_context(tc.tile_pool(name="consts", bufs=1))
        big = ctx.enter_context(tc.tile_pool(name="big", bufs=1))
        xq_pool = ctx.enter_context(tc.tile_pool(name="xq", bufs=1))
        x_pool = ctx.enter_context(tc.tile_pool(name="xs", bufs=3))
        pt_pool = ctx.enter_context(tc.tile_pool(name="pt", bufs=20))
        osb_pool = ctx.enter_context(tc.tile_pool(name="osb", bufs=4))
        rec_pool = ctx.enter_context(tc.tile_pool(name="rec", bufs=4))
        sp = ctx.enter_context(tc.tile_pool(name="sp", bufs=3, space="PSUM"))
        op = ctx.enter_context(tc.tile_pool(name="op", bufs=2, space="PSUM"))

        # PE warmup: dense matmuls on an undefined const tile with no DMA
        # dependency, so the HAM un-throttles while input DMA is in flight.
        warm_sb = consts.tile([128, 512], bf16)
        nc.vector.memset(warm_sb[:], 1.0)
        warm_ps = sp.tile([128, 512], f32, tag="sps")
        for _ in range(N_WARMUP):
            nc.tensor.matmul(
                warm_ps[:], lhsT=warm_sb[:, 0:128], rhs=warm_sb[:],
                start=True, stop=True,
            )

        w_sb = consts.tile([128, 3, 8, D], bf16)
        nc.sync.dma_start(w_sb[:], w_all)
        wq_sb, wk_sb, wv_sb = w_sb[:, 0], w_sb[:, 1], w_sb[:, 2]
        mask_sb = consts.tile([128, 1024], bf16)
        nc.sync.dma_start(mask_sb[:], mask)
        ident_sb = consts.tile([128, 128], bf16)
        make_identity(nc, ident_sb[:])

        qt_sb = big.tile([128, SQ], bf16)
        kt_sb = big.tile([128, S], bf16)
        v_sb = big.tile([128, 32, D + 1], bf16)
        nc.vector.memset(v_sb[:, :, D : D + 1], 1.0)

        def q_block(blk):  # Q_T columns [512*blk, 512*(blk+1))
            xq_t = xq_pool.tile([128, 8, 512], bf16)
            nc.sync.dma_start(xq_t[:], xqt[:, :, ds(blk * 512, 512)])
            ps = sp.tile([128, 512], f32, tag="sps")
            for e in range(8):
                nc.tensor.matmul(
                    ps[:],
                    lhsT=wq_sb[:, e, :],
                    rhs=xq_t[:, e, :],
                    start=(e == 0),
                    stop=(e == 7),
                )
            nc.vector.tensor_copy(qt_sb[:, ds(blk * 512, 512)], ps[:])

        def kv_block(blk):  # K_T columns and V rows [512*blk, 512*(blk+1))
            x_t = x_pool.tile([128, 8, 512], bf16)
            nc.sync.dma_start(x_t[:], xt[:, :, ds(blk * 512, 512)])
            ps = sp.tile([128, 512], f32, tag="sps")
            for e in range(8):
                nc.tensor.matmul(
                    ps[:],
                    lhsT=wk_sb[:, e, :],
                    rhs=x_t[:, e, :],
                    start=(e == 0),
                    stop=(e == 7),
                )
            nc.vector.tensor_copy(kt_sb[:, ds(blk * 512, 512)], ps[:])
            for st in range(4):
                psv = op.tile([128, D], f32, tag="ops")
                for e in range(8):
                    nc.tensor.matmul(
                        psv[:],
                        lhsT=x_t[:, e, ds(st * 128, 128)],
                        rhs=wv_sb[:, e, :],
                        start=(e == 0),
                        stop=(e == 7),
                    )
                nc.vector.tensor_copy(v_sb[:, blk * 4 + st, 0:D], psv[:])

        def attn_scores(m):
            # groups of 4 key tiles; the last group is masked and issued first
            # so its exp latency hides behind the remaining score matmuls.
            ngrp = m + 1
            order = [ngrp - 1] + list(range(ngrp - 1))
            pts = [None] * ngrp
            chunks = []
            for grp in order:
                masked = grp == ngrp - 1

                def emit(grp=grp, masked=masked):
                    sps = sp.tile([128, 1024], f32, tag="sps")
                    for i in range(4):
                        nc.tensor.matmul(
                            sps[:, ds(i * PAIR_W, PAIR_W)],
                            lhsT=kt_sb[:, ts(4 * grp + i, 128)],
                            rhs=qt_sb[:, ds(m * PAIR_W, PAIR_W)],
                            start=True,
                            stop=not masked,
                        )
                        if masked:
                            # Fold the causal mask in on the TensorEngine:
                            # accumulate identity.T @ mask into the still-open
                            # PSUM group; each quarter closes before the next.
                            nc.tensor.matmul(
                                sps[:, ds(i * PAIR_W, PAIR_W)],
                                lhsT=ident_sb[:],
                                rhs=mask_sb[:, ds(i * PAIR_W, PAIR_W)],
                                start=False,
                                stop=True,
                            )
                    pt = pt_pool.tile([128, 1024], bf16)
                    nc.scalar.activation(
                        pt[:],
                        sps[:],
                        func=mybir.ActivationFunctionType.Exp,
                        scale=SCALE,
                    )
                    pts[grp] = pt

                chunks.append(emit)
            return pts, chunks

        def attn_av_chunks(m, pts):
            # AV + softmax-denominator accumulation for pair m, as a list of
            # closures so the caller can interleave them with other PE work.
            chunks = []
            T = 4 * (m + 1)
            for c in range(2):
                ops = op.tile([128, D + 1], f32, tag="ops")

                def fin(c=c, ops=ops):
                    rc = rec_pool.tile([128, 1], f32)
                    nc.vector.reciprocal(rc[:], ops[:, D : D + 1])
                    osb = osb_pool.tile([128, D], f32)
                    nc.vector.tensor_scalar_mul(osb[:], ops[:, 0:D], rc[:])
                    nc.sync.dma_start(out[ts(2 * m + c, 128), :], osb[:])

                for t0 in range(0, T, 4):
                    def emit(c=c, ops=ops, t0=t0, last=(t0 + 4 >= T), fin=fin):
                        for t in range(t0, min(t0 + 4, T)):
                            nc.tensor.matmul(
                                ops[:],
                                lhsT=pts[t // 4][:, ds((t % 4) * PAIR_W + c * 128, 128)],
                                rhs=v_sb[:, t, :],
                                start=(t == 0),
                                stop=(t == T - 1),
                            )
                        if last:
                            fin()

                    chunks.append(emit)
            return chunks

        # Software pipeline: kv_block(m) produces the key tiles pair m needs;
        # pair m-1's AV matmuls are interleaved between pair m's score groups
        # so ACT exp latency hides behind PE work.
        prev_av = []
        for m in range(NPAIR):
            if m < 4:
                q_block(m)
            kv_block(m)
            pts, s_chunks = attn_scores(m)
            na, ns = len(prev_av), len(s_chunks)
            ai = 0
            for si, sc in enumerate(s_chunks):
                sc()
                want = (si + 1) * na // ns
                while ai < want:
                    prev_av[ai]()
                    ai += 1
            while ai < na:
                prev_av[ai]()
                ai += 1
            prev_av = attn_av_chunks(m, pts)
        for ch in prev_av:
            ch()

    nc.compile()
    return nc


def _qrows(g: int) -> np.ndarray:
    rows = np.empty(SQ, np.int64)
    for L in range(16):
        m, c = divmod(L, 2)
        a = 4 * m + 2 * g + c
        rows[L * 128 : (L + 1) * 128] = np.arange(a * 128, (a + 1) * 128)
    return rows


def _mask(g: int) -> np.ndarray:
    tri = np.where(
        np.arange(128)[:, None] <= np.arange(128)[None, :], 0.0, NEG
    ).astype(np.float32)
    M = np.zeros((4, 128, PAIR_W), np.float32)
    for r in range(4):
        for c in range(2):
            qa = 2 * g + c
            if r == qa:
                M[r, :, c * 128 : (c + 1) * 128] = tri
            elif r > qa:
                M[r, :, c * 128 : (c + 1) * 128] = NEG
    return (
        np.ascontiguousarray(M.transpose(1, 0, 2))
        .reshape(128, 1024)
        .astype(ml_dtypes.bfloat16)
    )


def _pack_w(w: np.ndarray) -> np.ndarray:
    bf = ml_dtypes.bfloat16
    return np.ascontiguousarray(
        np.asarray(w, np.float32).reshape(8, 128, D).transpose(1, 0, 2)
    ).astype(bf)


def _pack_xt(xT: np.ndarray) -> np.ndarray:
    # xT: [E, cols] float or bf16 -> [128, 8, cols]
    cols = xT.shape[1]
    return np.ascontiguousarray(xT.reshape(8, 128, cols).transpose(1, 0, 2))


def build_in_maps(x, Wq, Wk, Wv):
    bf = ml_dtypes.bfloat16
    x = np.asarray(x, np.float32)
    x16 = x.astype(bf)
    w_all = np.ascontiguousarray(
        np.stack([_pack_w(Wq), _pack_w(Wk), _pack_w(Wv)], axis=1)
    )
    masks = {g: _mask(g) for g in (0, 1)}
    qrows = {g: _qrows(g) for g in (0, 1)}

    in_maps = []
    for core in range(N_CORES):
        b, g = divmod(core, 2)
        xTb = np.ascontiguousarray(x16[b].T)  # [E, S] bf16
        in_maps.append(
            {
                "xt": _pack_xt(xTb),
                "xqt": _pack_xt(np.ascontiguousarray(xTb[:, qrows[g]])),
                "w_all": w_all,
                "mask": masks[g],
            }
        )
    return in_maps


def kernel(x, Wq, Wk, Wv):
    _install_ntff_shim()
    from concourse.bass_utils import run_bass_kernel_spmd

    if "nc" not in _cache:
        _cache["nc"] = build_nc()
    nc = _cache["nc"]

    in_maps = build_in_maps(x, Wq, Wk, Wv)
    res = run_bass_kernel_spmd(nc, in_maps, core_ids=list(range(N_CORES)))
    qrows = {g: _qrows(g) for g in (0, 1)}
    out = np.empty((B, S, D), np.float32)
    for core in range(N_CORES):
        b, g = divmod(core, 2)
        out[b][qrows[g]] = res.results[core]["out"]
    return out


# revision 13
# speedup vs baseline: 1.0422x; 1.0422x over previous
"""Causal single-head attention (B=4, S=4096, E=1024, D=128) on 8 TRN2 NeuronCores.

Sharding: core = (batch b, query-group g) with b = core//2, g = core%2.
Each core owns batch b and half of its query tiles, interleaved in pairs of
128-row tiles so causal work stays balanced: local pair m covers absolute
query tiles {4m+2g, 4m+2g+1} and attends to key tiles 0..4m+3.

Per-core device program (bf16 matmuls, fp32 accumulate):
  Q_T = Wq.T @ xq.T   [d=128, 2048]   (lhsT = Wq e-chunks, rhs = xqT)
  K_T = Wk.T @ x.T    [d=128, 4096]
  V   = x @ Wv        [4096, 128] as 32 partition-tiles, plus a ones column
                      so the attention matmul also accumulates the softmax
                      denominator l = sum_k exp(s).
  per pair m, key-tile group grp in 0..m (4 key tiles per group):
      s_T[4 tiles] -> one [128, 1024] PSUM supertile  (4 matmuls, N=256)
      += causal mask on the last group (per-core input: the same graph slot
         needs different masks per query-group)
      P_T = exp(s_T * D**-0.5) in one activation  (no row-max: scores ~N(0,1))
  per query tile c in pair:
      O = sum_t P_T[t][:,c].T @ [V[t] | 1]       [q=128, 129]    (PSUM)
      out[q,:] = O[:,0:128] * (1 / O[:,128])
"""

import sys
import types

import numpy as np
import ml_dtypes

B, S, E, D = 4, 4096, 1024, 128
N_CORES = 8
NPAIR = 8          # local query-tile pairs per core
PAIR_W = 256       # two 128-row query tiles
SQ = S // 2        # query rows per core
SCALE = float(D) ** -0.5
NEG = -1e30
N_WARMUP = 30      # PE warmup matmuls issued before any DMA-dependent work

_cache = {}


def _install_ntff_shim():
    """antenv.axon_hooks is missing in this image; recreate it so
    run_bass_kernel_spmd(trace=True) can capture NTFF profiles."""
    if "antenv.axon_hooks" in sys.modules:
        return
    try:
        from trn_agent_boot.trn_boot import _ntff_profile_via_ctypes

        hook = _ntff_profile_via_ctypes("/opt/axon/libaxon_pjrt.so")
    except Exception:
        hook = None
    mod = types.ModuleType("antenv.axon_hooks")
    mod.get_axon_ntff_profile_hook = lambda: hook
    mod.set_axon_ntff_profile_hook = lambda h: None
    sys.modules["antenv.axon_hooks"] = mod


def build_nc():
    from contextlib import ExitStack

    import concourse.tile as tile
    from concourse import bacc, mybir
    from concourse.bass import ds, ts
    from concourse.masks import make_identity

    f32 = mybir.dt.float32
    bf16 = mybir.dt.bfloat16

    nc = bacc.Bacc("TRN2", target_bir_lowering=False, debug=False)
    xt = nc.dram_tensor("xt", [128, 8, S], bf16, kind="ExternalInput").ap()
    xqt = nc.dram_tensor("xqt", [128, 8, SQ], bf16, kind="ExternalInput").ap()
    w_all = nc.dram_tensor("w_all", [128, 3, 8, D], bf16, kind="ExternalInput").ap()
    mask = nc.dram_tensor("mask", [128, 1024], bf16, kind="ExternalInput").ap()
    out = nc.dram_tensor("out", [SQ, D], f32, kind="ExternalOutput").ap()

    with tile.TileContext(nc) as tc, ExitStack() as ctx:
        consts = ctx.enter_context(tc.tile_pool(name="consts", bufs=1))
        big = ctx.enter_context(tc.tile_pool(name="big", bufs=1))
        xq_pool = ctx.enter_context(tc.tile_pool(name="xq", bufs=1))
        x_pool = ctx.enter_context(tc.tile_pool(name="xs", bufs=2))
        pt_pool = ctx.enter_context(tc.tile_pool(name="pt", bufs=20))
        osb_pool = ctx.enter_context(tc.tile_pool(name="osb", bufs=4))
        rec_pool = ctx.enter_context(tc.tile_pool(name="rec", bufs=4))
        sp = ctx.enter_context(tc.tile_pool(name="sp", bufs=3, space="PSUM"))
        op = ctx.enter_context(tc.tile_pool(name="op", bufs=2, space="PSUM"))

        # PE warmup: dense matmuls on an undefined const tile with no DMA
        # dependency, so the HAM un-throttles while input DMA is in flight.
        warm_sb = consts.tile([128, 512], bf16)
        nc.vector.memset(warm_sb[:], 1.0)
        warm_ps = sp.tile([128, 512], f32, tag="sps")
        for _ in range(N_WARMUP):
            nc.tensor.matmul(
                warm_ps[:], lhsT=warm_sb[:, 0:128], rhs=warm_sb[:],
                start=True, stop=True,
            )

        w_sb = consts.tile([128, 3, 8, D], bf16)
        nc.sync.dma_start(w_sb[:], w_all)
        wq_sb, wk_sb, wv_sb = w_sb[:, 0], w_sb[:, 1], w_sb[:, 2]
        mask_sb = consts.tile([128, 1024], bf16)
        ident_sb = consts.tile([128, 128], bf16)
        make_identity(nc, ident_sb[:])

        qt_sb = big.tile([128, SQ], bf16)
        kt_sb = big.tile([128, S], bf16)
        v_sb = big.tile([128, 32, D + 1], bf16)
        nc.vector.memset(v_sb[:, :, D : D + 1], 1.0)

        def q_block(blk):  # Q_T columns [512*blk, 512*(blk+1))
            xq_t = xq_pool.tile([128, 8, 512], bf16)
            nc.sync.dma_start(xq_t[:], xqt[:, :, ds(blk * 512, 512)])
            ps = sp.tile([128, 512], f32, tag="sps")
            for e in range(8):
                nc.tensor.matmul(
                    ps[:],
                    lhsT=wq_sb[:, e, :],
                    rhs=xq_t[:, e, :],
                    start=(e == 0),
                    stop=(e == 7),
                )
            nc.vector.tensor_copy(qt_sb[:, ds(blk * 512, 512)], ps[:])

        def kv_block(blk):  # K_T columns and V rows [512*blk, 512*(blk+1))
            x_t = x_pool.tile([128, 8, 512], bf16)
            nc.sync.dma_start(x_t[:], xt[:, :, ds(blk * 512, 512)])
            ps = sp.tile([128, 512], f32, tag="sps")
            for e in range(8):
                nc.tensor.matmul(
                    ps[:],
                    lhsT=wk_sb[:, e, :],
                    rhs=x_t[:, e, :],
                    start=(e == 0),
                    stop=(e == 7),
                )
            nc.vector.tensor_copy(kt_sb[:, ds(blk * 512, 512)], ps[:])
            for st in range(4):
                psv = op.tile([128, D], f32, tag="ops")
                for e in range(8):
                    nc.tensor.matmul(
                        psv[:],
                        lhsT=x_t[:, e, ds(st * 128, 128)],
                        rhs=wv_sb[:, e, :],
                        start=(e == 0),
                        stop=(e == 7),
                    )
                nc.vector.tensor_copy(v_sb[:, blk * 4 + st, 0:D], psv[:])

        def attn_scores(m):
            # groups of 4 key tiles; the last group is masked and issued first
            # so its exp latency hides behind the remaining score matmuls.
            ngrp = m + 1
            order = [ngrp - 1] + list(range(ngrp - 1))
            pts = [None] * ngrp
            chunks = []
            for grp in order:
                masked = grp == ngrp - 1

                def emit(grp=grp, masked=masked):
                    sps = sp.tile([128, 1024], f32, tag="sps")
                    for i in range(4):
                        nc.tensor.matmul(
                            sps[:, ds(i * PAIR_W, PAIR_W)],
                            lhsT=kt_sb[:, ts(4 * grp + i, 128)],
                            rhs=qt_sb[:, ds(m * PAIR_W, PAIR_W)],
                            start=True,
                            stop=not masked,
                        )
                        if masked:
                            # Fold the causal mask in on the TensorEngine:
                            # accumulate identity.T @ mask into the still-open
                            # PSUM group; each quarter closes before the next.
                            nc.tensor.matmul(
                                sps[:, ds(i * PAIR_W, PAIR_W)],
                                lhsT=ident_sb[:],
                                rhs=mask_sb[:, ds(i * PAIR_W, PAIR_W)],
                                start=False,
                                stop=True,
                            )
                    pt = pt_pool.tile([128, 1024], bf16)
                    nc.scalar.activation(
                        pt[:],
                        sps[:],
                        func=mybir.ActivationFunctionType.Exp,
                        scale=SCALE,
                    )
                    pts[grp] = pt

                chunks.append(emit)
            return pts, chunks

        def attn_av_chunks(m, pts):
            # AV + softmax-denominator accumulation for pair m, as a list of
            # closures so the caller can interleave them with other PE work.
            chunks = []
            T = 4 * (m + 1)
            for c in range(2):
                ops = op.tile([128, D + 1], f32, tag="ops")

                def fin(c=c, ops=ops):
                    rc = rec_pool.tile([128, 1], f32)
                    nc.vector.reciprocal(rc[:], ops[:, D : D + 1])
                    osb = osb_pool.tile([128, D], f32)
                    nc.vector.tensor_scalar_mul(osb[:], ops[:, 0:D], rc[:])
                    nc.sync.dma_start(out[ts(2 * m + c, 128), :], osb[:])

                for t0 in range(0, T, 4):
                    def emit(c=c, ops=ops, t0=t0, last=(t0 + 4 >= T), fin=fin):
                        for t in range(t0, min(t0 + 4, T)):
                            nc.tensor.matmul(
                                ops[:],
                                lhsT=pts[t // 4][:, ds((t % 4) * PAIR_W + c * 128, 128)],
                                rhs=v_sb[:, t, :],
                                start=(t == 0),
                                stop=(t == T - 1),
                            )
                        if last:
                            fin()

                    chunks.append(emit)
            return chunks

        # Software pipeline: kv_block(m) produces the key tiles pair m needs;
        # pair m-1's AV matmuls are interleaved between pair m's score groups
        # so ACT exp latency hides behind PE work.
        prev_av = []
        for m in range(NPAIR):
            if m < 4:
                q_block(m)
            kv_block(m)
            if m == 0:
                # deferred so it doesn't compete with the first x DMA blocks
                nc.sync.dma_start(mask_sb[:], mask)
            pts, s_chunks = attn_scores(m)
            na, ns = len(prev_av), len(s_chunks)
            ai = 0
            for si, sc in enumerate(s_chunks):
                sc()
                want = (si + 1) * na // ns
                while ai < want:
                    prev_av[ai]()
                    ai += 1
            while ai < na:
                prev_av[ai]()
                ai += 1
            prev_av = attn_av_chunks(m, pts)
        for ch in prev_av:
            ch()

    nc.compile()
    return nc


def _qrows(g: int) -> np.ndarray:
    rows = np.empty(SQ, np.int64)
    for L in range(16):
        m, c = divmod(L, 2)
        a = 4 * m + 2 * g + c
        rows[L * 128 : (L + 1) * 128] = np.arange(a * 128, (a + 1) * 128)
    return rows


def _mask(g: int) -> np.ndarray:
    tri = np.where(
        np.arange(128)[:, None] <= np.arange(128)[None, :], 0.0, NEG
    ).astype(np.float32)
    M = np.zeros((4, 128, PAIR_W), np.float32)
    for r in range(4):
        for c in range(2):
            qa = 2 * g + c
            if r == qa:
                M[r, :, c * 128 : (c + 1) * 128] = tri
            elif r > qa:
                M[r, :, c * 128 : (c + 1) * 128] = NEG
    return (
        np.ascontiguousarray(M.transpose(1, 0, 2))
        .reshape(128, 1024)
        .astype(ml_dtypes.bfloat16)
    )


def _pack_w(w: np.ndarray) -> np.ndarray:
    bf = ml_dtypes.bfloat16
    return np.ascontiguousarray(
        np.asarray(w, np.float32).reshape(8, 128, D).transpose(1, 0, 2)
    ).astype(bf)


def _pack_xt(xT: np.ndarray) -> np.ndarray:
    # xT: [E, cols] float or bf16 -> [128, 8, cols]
    cols = xT.shape[1]
    return np.ascontiguousarray(xT.reshape(8, 128, cols).transpose(1, 0, 2))


def build_in_maps(x, Wq, Wk, Wv):
    bf = ml_dtypes.bfloat16
    x = np.asarray(x, np.float32)
    x16 = x.astype(bf)
    w_all = np.ascontiguousarray(
        np.stack([_pack_w(Wq), _pack_w(Wk), _pack_w(Wv)], axis=1)
    )
    masks = {g: _mask(g) for g in (0, 1)}
    qrows = {g: _qrows(g) for g in (0, 1)}

    in_maps = []
    for core in range(N_CORES):
        b, g = divmod(core, 2)
        xTb = np.ascontiguousarray(x16[b].T)  # [E, S] bf16
        in_maps.append(
            {
                "xt": _pack_xt(xTb),
                "xqt": _pack_xt(np.ascontiguousarray(xTb[:, qrows[g]])),
                "w_all": w_all,
                "mask": masks[g],
            }
        )
    return in_maps


def kernel(x, Wq, Wk, Wv):
    _install_ntff_shim()
    from concourse.bass_utils import run_bass_kernel_spmd

    if "nc" not in _cache:
        _cache["nc"] = build_nc()
    nc = _cache["nc"]

    in_maps = build_in_maps(x, Wq, Wk, Wv)
    res = run_bass_kernel_spmd(nc, in_maps, core_ids=list(range(N_CORES)))
    qrows = {g: _qrows(g) for g in (0, 1)}
    out = np.empty((B, S, D), np.float32)
    for core in range(N_CORES):
        b, g = divmod(core, 2)
        out[b][qrows[g]] = res.results[core]["out"]
    return out


# revision 14
# speedup vs baseline: 1.0515x; 1.0089x over previous
"""Causal single-head attention (B=4, S=4096, E=1024, D=128) on 8 TRN2 NeuronCores.

Sharding: core = (batch b, query-group g) with b = core//2, g = core%2.
Each core owns batch b and half of its query tiles, interleaved in pairs of
128-row tiles so causal work stays balanced: local pair m covers absolute
query tiles {4m+2g, 4m+2g+1} and attends to key tiles 0..4m+3.

Per-core device program (bf16 matmuls, fp32 accumulate):
  Q_T = Wq.T @ xq.T   [d=128, 2048]   (lhsT = Wq e-chunks, rhs = xqT)
  K_T = Wk.T @ x.T    [d=128, 4096]
  V   = x @ Wv        [4096, 128] as 32 partition-tiles, plus a ones column
                      so the attention matmul also accumulates the softmax
                      denominator l = sum_k exp(s).
  per pair m, key-tile group grp in 0..m (4 key tiles per group):
      s_T[4 tiles] -> one [128, 1024] PSUM supertile  (4 matmuls, N=256)
      += causal mask on the last group (per-core input: the same graph slot
         needs different masks per query-group)
      P_T = exp(s_T * D**-0.5) in one activation  (no row-max: scores ~N(0,1))
  per query tile c in pair:
      O = sum_t P_T[t][:,c].T @ [V[t] | 1]       [q=128, 129]    (PSUM)
      out[q,:] = O[:,0:128] * (1 / O[:,128])
"""

import sys
import types

import numpy as np
import ml_dtypes

B, S, E, D = 4, 4096, 1024, 128
N_CORES = 8
NPAIR = 8          # local query-tile pairs per core
PAIR_W = 256       # two 128-row query tiles
SQ = S // 2        # query rows per core
SCALE = float(D) ** -0.5
NEG = -1e30
N_WARMUP = 30      # PE warmup matmuls issued before any DMA-dependent work

_cache = {}


def _install_ntff_shim():
    """antenv.axon_hooks is missing in this image; recreate it so
    run_bass_kernel_spmd(trace=True) can capture NTFF profiles."""
    if "antenv.axon_hooks" in sys.modules:
        return
    try:
        from trn_agent_boot.trn_boot import _ntff_profile_via_ctypes

        hook = _ntff_profile_via_ctypes("/opt/axon/libaxon_pjrt.so")
    except Exception:
        hook = None
    mod = types.ModuleType("antenv.axon_hooks")
    mod.get_axon_ntff_profile_hook = lambda: hook
    mod.set_axon_ntff_profile_hook = lambda h: None
    sys.modules["antenv.axon_hooks"] = mod


def build_nc():
    from contextlib import ExitStack

    import concourse.tile as tile
    from concourse import bacc, mybir
    from concourse.bass import ds, ts

    f32 = mybir.dt.float32
    bf16 = mybir.dt.bfloat16

    nc = bacc.Bacc("TRN2", target_bir_lowering=False, debug=False)
    xt = nc.dram_tensor("xt", [128, 8, S], bf16, kind="ExternalInput").ap()
    xqt = nc.dram_tensor("xqt", [128, 8, SQ], bf16, kind="ExternalInput").ap()
    w_all = nc.dram_tensor("w_all", [128, 3, 8, D], bf16, kind="ExternalInput").ap()
    mask = nc.dram_tensor("mask", [128, 1024], bf16, kind="ExternalInput").ap()
    out = nc.dram_tensor("out", [SQ, D], f32, kind="ExternalOutput").ap()

    with tile.TileContext(nc) as tc, ExitStack() as ctx:
        consts = ctx.enter_context(tc.tile_pool(name="consts", bufs=1))
        big = ctx.enter_context(tc.tile_pool(name="big", bufs=1))
        xq_pool = ctx.enter_context(tc.tile_pool(name="xq", bufs=1))
        x_pool = ctx.enter_context(tc.tile_pool(name="xs", bufs=2))
        pt_pool = ctx.enter_context(tc.tile_pool(name="pt", bufs=20))
        osb_pool = ctx.enter_context(tc.tile_pool(name="osb", bufs=4))
        rec_pool = ctx.enter_context(tc.tile_pool(name="rec", bufs=4))
        sp = ctx.enter_context(tc.tile_pool(name="sp", bufs=3, space="PSUM"))
        op = ctx.enter_context(tc.tile_pool(name="op", bufs=2, space="PSUM"))

        # PE warmup: dense matmuls on an undefined const tile with no DMA
        # dependency, so the HAM un-throttles while input DMA is in flight.
        warm_sb = consts.tile([128, 512], bf16)
        nc.vector.memset(warm_sb[:], 1.0)
        warm_ps = sp.tile([128, 512], f32, tag="sps")
        for _ in range(N_WARMUP):
            nc.tensor.matmul(
                warm_ps[:], lhsT=warm_sb[:, 0:128], rhs=warm_sb[:],
                start=True, stop=True,
            )

        w_sb = consts.tile([128, 3, 8, D], bf16)
        nc.sync.dma_start(w_sb[:], w_all)
        wq_sb, wk_sb, wv_sb = w_sb[:, 0], w_sb[:, 1], w_sb[:, 2]
        mask_sb = consts.tile([128, 1024], bf16)

        qt_sb = big.tile([128, SQ], bf16)
        kt_sb = big.tile([128, S], bf16)
        v_sb = big.tile([128, 32, D + 1], bf16)
        nc.vector.memset(v_sb[:, :, D : D + 1], 1.0)

        def q_block(blk):  # Q_T columns [512*blk, 512*(blk+1))
            xq_t = xq_pool.tile([128, 8, 512], bf16)
            nc.sync.dma_start(xq_t[:], xqt[:, :, ds(blk * 512, 512)])
            ps = sp.tile([128, 512], f32, tag="sps")
            for e in range(8):
                nc.tensor.matmul(
                    ps[:],
                    lhsT=wq_sb[:, e, :],
                    rhs=xq_t[:, e, :],
                    start=(e == 0),
                    stop=(e == 7),
                )
            nc.vector.tensor_copy(qt_sb[:, ds(blk * 512, 512)], ps[:])

        def kv_block(blk):  # K_T columns and V rows [512*blk, 512*(blk+1))
            x_t = x_pool.tile([128, 8, 512], bf16)
            nc.sync.dma_start(x_t[:], xt[:, :, ds(blk * 512, 512)])
            ps = sp.tile([128, 512], f32, tag="sps")
            for e in range(8):
                nc.tensor.matmul(
                    ps[:],
                    lhsT=wk_sb[:, e, :],
                    rhs=x_t[:, e, :],
                    start=(e == 0),
                    stop=(e == 7),
                )
            nc.vector.tensor_copy(kt_sb[:, ds(blk * 512, 512)], ps[:])
            for st in range(4):
                psv = op.tile([128, D], f32, tag="ops")
                for e in range(8):
                    nc.tensor.matmul(
                        psv[:],
                        lhsT=x_t[:, e, ds(st * 128, 128)],
                        rhs=wv_sb[:, e, :],
                        start=(e == 0),
                        stop=(e == 7),
                    )
                nc.vector.tensor_copy(v_sb[:, blk * 4 + st, 0:D], psv[:])

        def attn_scores(m):
            # groups of 4 key tiles; the last group is masked and issued first
            # so its exp latency hides behind the remaining score matmuls.
            ngrp = m + 1
            order = [ngrp - 1] + list(range(ngrp - 1))
            pts = [None] * ngrp
            chunks = []
            for grp in order:
                masked = grp == ngrp - 1

                def emit(grp=grp, masked=masked):
                    sps = sp.tile([128, 1024], f32, tag="sps")
                    for i in range(4):
                        nc.tensor.matmul(
                            sps[:, ds(i * PAIR_W, PAIR_W)],
                            lhsT=kt_sb[:, ts(4 * grp + i, 128)],
                            rhs=qt_sb[:, ds(m * PAIR_W, PAIR_W)],
                            start=True,
                            stop=True,
                        )
                    pt = pt_pool.tile([128, 1024], bf16)
                    nc.scalar.activation(
                        pt[:],
                        sps[:],
                        func=mybir.ActivationFunctionType.Exp,
                        scale=SCALE,
                    )
                    if masked:
                        # Causal mask as a 0/1 multiply on the exp'd tile
                        # (DVE bf16 4x mode; unmasked scores are O(1) so the
                        # pre-mask exp can't overflow).
                        nc.vector.tensor_mul(pt[:], pt[:], mask_sb[:])
                    pts[grp] = pt

                chunks.append(emit)
            return pts, chunks

        def attn_av_chunks(m, pts):
            # AV + softmax-denominator accumulation for pair m, as a list of
            # closures so the caller can interleave them with other PE work.
            chunks = []
            T = 4 * (m + 1)
            for c in range(2):
                ops = op.tile([128, D + 1], f32, tag="ops")

                def fin(c=c, ops=ops):
                    rc = rec_pool.tile([128, 1], f32)
                    nc.vector.reciprocal(rc[:], ops[:, D : D + 1])
                    osb = osb_pool.tile([128, D], f32)
                    nc.vector.tensor_scalar_mul(osb[:], ops[:, 0:D], rc[:])
                    nc.sync.dma_start(out[ts(2 * m + c, 128), :], osb[:])

                for t0 in range(0, T, 4):
                    def emit(c=c, ops=ops, t0=t0, last=(t0 + 4 >= T), fin=fin):
                        for t in range(t0, min(t0 + 4, T)):
                            nc.tensor.matmul(
                                ops[:],
                                lhsT=pts[t // 4][:, ds((t % 4) * PAIR_W + c * 128, 128)],
                                rhs=v_sb[:, t, :],
                                start=(t == 0),
                                stop=(t == T - 1),
                            )
                        if last:
                            fin()

                    chunks.append(emit)
            return chunks

        # Software pipeline: kv_block(m) produces the key tiles pair m needs;
        # pair m-1's AV matmuls are interleaved between pair m's score groups
        # so ACT exp latency hides behind PE work.
        prev_av = []
        for m in range(NPAIR):
            if m < 4:
                q_block(m)
            kv_block(m)
            if m == 0:
                # deferred so it doesn't compete with the first x DMA blocks
                nc.sync.dma_start(mask_sb[:], mask)
            pts, s_chunks = attn_scores(m)
            na, ns = len(prev_av), len(s_chunks)
            ai = 0
            for si, sc in enumerate(s_chunks):
                sc()
                want = (si + 1) * na // ns
                while ai < want:
                    prev_av[ai]()
                    ai += 1
            while ai < na:
                prev_av[ai]()
                ai += 1
            prev_av = attn_av_chunks(m, pts)
        for ch in prev_av:
            ch()

    nc.compile()
    return nc


def _qrows(g: int) -> np.ndarray:
    rows = np.empty(SQ, np.int64)
    for L in range(16):
        m, c = divmod(L, 2)
        a = 4 * m + 2 * g + c
        rows[L * 128 : (L + 1) * 128] = np.arange(a * 128, (a + 1) * 128)
    return rows


def _mask(g: int) -> np.ndarray:
    tri = (np.arange(128)[:, None] <= np.arange(128)[None, :]).astype(np.float32)
    M = np.ones((4, 128, PAIR_W), np.float32)
    for r in range(4):
        for c in range(2):
            qa = 2 * g + c
            if r == qa:
                M[r, :, c * 128 : (c + 1) * 128] = tri
            elif r > qa:
                M[r, :, c * 128 : (c + 1) * 128] = 0.0
    return (
        np.ascontiguousarray(M.transpose(1, 0, 2))
        .reshape(128, 1024)
        .astype(ml_dtypes.bfloat16)
    )


def _pack_w(w: np.ndarray) -> np.ndarray:
    bf = ml_dtypes.bfloat16
    return np.ascontiguousarray(
        np.asarray(w, np.float32).reshape(8, 128, D).transpose(1, 0, 2)
    ).astype(bf)


def _pack_xt(xT: np.ndarray) -> np.ndarray:
    # xT: [E, cols] float or bf16 -> [128, 8, cols]
    cols = xT.shape[1]
    return np.ascontiguousarray(xT.reshape(8, 128, cols).transpose(1, 0, 2))


def build_in_maps(x, Wq, Wk, Wv):
    bf = ml_dtypes.bfloat16
    x = np.asarray(x, np.float32)
    x16 = x.astype(bf)
    w_all = np.ascontiguousarray(
        np.stack([_pack_w(Wq), _pack_w(Wk), _pack_w(Wv)], axis=1)
    )
    masks = {g: _mask(g) for g in (0, 1)}
    qrows = {g: _qrows(g) for g in (0, 1)}

    in_maps = []
    for core in range(N_CORES):
        b, g = divmod(core, 2)
        xTb = np.ascontiguousarray(x16[b].T)  # [E, S] bf16
        in_maps.append(
            {
                "xt": _pack_xt(xTb),
                "xqt": _pack_xt(np.ascontiguousarray(xTb[:, qrows[g]])),
                "w_all": w_all,
                "mask": masks[g],
            }
        )
    return in_maps


def kernel(x, Wq, Wk, Wv):
    _install_ntff_shim()
    from concourse.bass_utils import run_bass_kernel_spmd

    if "nc" not in _cache:
        _cache["nc"] = build_nc()
    nc = _cache["nc"]

    in_maps = build_in_maps(x, Wq, Wk, Wv)
    res = run_bass_kernel_spmd(nc, in_maps, core_ids=list(range(N_CORES)))
    qrows = {g: _qrows(g) for g in (0, 1)}
    out = np.empty((B, S, D), np.float32)
    for core in range(N_CORES):
        b, g = divmod(core, 2)
        out[b][qrows[g]] = res.results[core]["out"]
    return out
